# revision 19
# baseline (speedup 1.0000x reference)
"""Trainium2 Bass kernel for KAN([128,128,128]) + cosine-sim VQ codebook assignment.

Math: each KANLinear layer = silu(x) @ base_w.T + einsum('nic,oic->no', b_splines(x), sw).
The cubic B-spline einsum is folded into plain matmuls via the truncated-power
representation  B(u) = (1/6) sum_r (-1)^r C(4,r) relu(u-r)^3,  applied two-sided
(left rep for bases 4..7, right rep for bases 0..3) with t = 2.5x+5.5 clamped to
[0,11] so the tails cancel exactly (integer cancellation). Each layer is then 15
K=128 matmuls over moving tiles built from shared squares (t-j)^2:
  silu, { d_j^3, relu_j*sq_j } for j=4..7,  relu_j*sq_j j=8..10,  min_j*sq_j j=1..3
with host-folded stationary weights.

Layout: feature-major (x^T: features on partitions, tokens on free). x is
pre-transposed on the host; emb is produced transposed and fixed on the host.
Per-512-token chunk: DMA x^T -> elementwise (DVE/ACT/GPSIMD) -> 15 MMs -> h^T ->
repeat -> emb^T -> DMA out; per-128: sim = emb^T.T @ pnT, argmax via
reduce_max / is_equal / iota / reduce_min.

Sharding: pure data-parallel over 8 cores, 16384 tokens each, weights replicated.
"""
import os
import sys

import numpy as np

for _p in ("/opt/trn_rl_repo", "/root/.axon_site/_ro/trn_rl_repo"):
    if _p not in sys.path and os.path.isdir(_p):
        sys.path.append(_p)


def _ensure_ntff_hook():
    """The agent image's antenv lacks axon_hooks; synthesize it and register
    the ctypes NTFF hook so trace=True works. Safe no-op on failure."""
    import types
    try:
        import antenv.axon_hooks  # noqa: F401
        return
    except ImportError:
        pass
    try:
        import antenv
        mod = types.ModuleType("antenv.axon_hooks")
        _state = {"hook": None}
        mod.set_axon_ntff_profile_hook = lambda h: _state.__setitem__("hook", h)
        mod.get_axon_ntff_profile_hook = lambda: _state["hook"]
        sys.modules["antenv.axon_hooks"] = mod
        antenv.axon_hooks = mod
        from trn_agent_boot.trn_boot import _ntff_profile_via_ctypes
        so = "/opt/axon/libaxon_pjrt.so"
        if os.path.exists(so):
            mod.set_axon_ntff_profile_hook(_ntff_profile_via_ctypes(so))
    except Exception:
        pass


_ensure_ntff_hook()

import concourse.bass as bass  # noqa: E402
import concourse.bacc as bacc  # noqa: E402
import concourse.mybir as mybir  # noqa: E402
from concourse import tile  # noqa: E402
from concourse.bass_utils import run_bass_kernel_spmd  # noqa: E402

F32 = mybir.dt.float32
I32 = mybir.dt.int32
AF = mybir.ActivationFunctionType
OP = mybir.AluOpType

N_TOTAL = 131072
N_CORES = 8
H = 128
K = 256
CHUNK = 512
BIG = 65536.0

SHARED_J = [4, 5, 6, 7]   # both R_j and L_j used -> tiles {d^3, relu*sq}
LEFT_ONLY = [8, 9, 10]    # relu(t-j)^3
RIGHT_ONLY = [1, 2, 3]    # (min(t-j,0))^3 = -relu(j-t)^3
N_MOV = 1 + 2 * len(SHARED_J) + len(LEFT_ONLY) + len(RIGHT_ONLY)  # 15
SILU_VIA_SIGMOID = False  # True: emulate silu as x*sigmoid(x) (for CoreSim)

_A4 = np.array([1.0, -4.0, 6.0, -4.0, 1.0])


def _fold_weights(base_w, spline_w, scaler):
    """Fold spline weights into per-moving-tile stationary blocks.

    Moving tiles (order): silu; for j in 4..7: d_j^3 then relu_j*sq_j;
    R_j for j=8..10; M_j (=min^3) for j=1..3.
    Returns (N_MOV, H, H) fp32, each block transposed to (i, o) = lhsT.
    """
    sww = spline_w.astype(np.float64) * scaler.astype(np.float64)[..., None]
    WR = np.zeros((12, H, H))  # weight on R_j = relu(t-j)^3, [o, i]
    WM = np.zeros((12, H, H))  # weight on M_j = (min(t-j,0))^3
    for c in range(4, 8):
        for r in range(5):
            WR[c + r] += sww[:, :, c] * (_A4[r] / 6.0)
    for c in range(0, 4):
        for r in range(5):
            # B_c = sum_r a_r/6 * L_{c+4-r},  L_j = -M_j
            WM[c + 4 - r] -= sww[:, :, c] * (_A4[r] / 6.0)
    blocks = [base_w.astype(np.float64)]
    for j in SHARED_J:
        # WR*R + WM*M = WM*d3 + (WR-WM)*R   (d3 = R + M)
        blocks.append(WM[j])
        blocks.append(WR[j] - WM[j])
    blocks += [WR[j] for j in LEFT_ONLY]
    blocks += [WM[j] for j in RIGHT_ONLY]
    return np.stack([b.T for b in blocks]).astype(np.float32)


def build_kernel(n_core: int):
    nc = bacc.Bacc("TRN2", target_bir_lowering=False, debug=False,
                   enable_asserts=False, num_devices=N_CORES)

    xt_d = nc.dram_tensor("xt", [H, n_core], F32, kind="ExternalInput")
    w1_d = nc.dram_tensor("w1", [N_MOV, H, H], F32, kind="ExternalInput")
    w2_d = nc.dram_tensor("w2", [N_MOV, H, H], F32, kind="ExternalInput")
    pnt_d = nc.dram_tensor("pnt", [H, K], F32, kind="ExternalInput")
    iotab_d = nc.dram_tensor("iotab", [H, K], F32, kind="ExternalInput")
    jbias_d = nc.dram_tensor("jbias", [H, 12], F32, kind="ExternalInput")
    embt_d = nc.dram_tensor("embt", [H, n_core], F32, kind="ExternalOutput")
    asn_d = nc.dram_tensor("asn", [n_core, 1], I32, kind="ExternalOutput")

    n_chunks = n_core // CHUNK
    QQ = CHUNK // H
    with tile.TileContext(nc) as tc:
        with (
            tc.tile_pool(name="const", bufs=1) as cpool,
            tc.tile_pool(name="xin", bufs=3) as xpool,
            tc.tile_pool(name="ew", bufs=3) as ewpool,
            tc.tile_pool(name="sqp", bufs=12) as sqpool,
            tc.tile_pool(name="rs", bufs=5) as rspool,
            tc.tile_pool(name="cube", bufs=2 * N_MOV) as cubepool,
            tc.tile_pool(name="et", bufs=2) as etpool,
            tc.tile_pool(name="aq", bufs=4) as aqpool,
            tc.tile_pool(name="small", bufs=8) as spool,
            tc.tile_pool(name="psB", bufs=2, space="PSUM") as psB,
            tc.tile_pool(name="psE", bufs=2, space="PSUM") as psE,
            tc.tile_pool(name="psS", bufs=3, space="PSUM") as psS,
        ):
            w1_sb = cpool.tile([H, N_MOV, H], F32)
            w2_sb = cpool.tile([H, N_MOV, H], F32)
            nc.sync.dma_start(w1_sb[:], w1_d.ap().rearrange("g i o -> i g o"))
            nc.sync.dma_start(w2_sb[:], w2_d.ap().rearrange("g i o -> i g o"))
            pnt_sb = cpool.tile([H, K], F32)
            nc.sync.dma_start(pnt_sb[:], pnt_d[:])
            iotab_sb = cpool.tile([H, K], F32)
            nc.sync.dma_start(iotab_sb[:], iotab_d[:])
            jbias_sb = cpool.tile([H, 12], F32)
            nc.sync.dma_start(jbias_sb[:], jbias_d[:])

            def kan_layer(inT, out_ps, w_sb, mulsplit):
                """inT: (H, CHUNK) feature-major input (SBUF or PSUM)."""
                silu = ewpool.tile([H, CHUNK], F32, tag="silu")
                if SILU_VIA_SIGMOID:
                    sig = ewpool.tile([H, CHUNK], F32, tag="sig")
                    nc.scalar.activation(sig[:], inT[:], AF.Sigmoid)
                    nc.vector.tensor_tensor(silu[:], sig[:], inT[:], OP.mult)
                else:
                    nc.scalar.activation(silu[:], inT[:], AF.Silu)
                nc.tensor.matmul(out_ps[:], w_sb[:, 0, :], silu[:],
                                 start=True, stop=False)
                t = ewpool.tile([H, CHUNK], F32, tag="t")
                nc.vector.tensor_scalar(t[:], inT[:], 2.5, 5.5, OP.mult, OP.add)
                tc_ = ewpool.tile([H, CHUNK], F32, tag="tc")
                nc.vector.tensor_scalar(tc_[:], t[:], 0.0, 11.0, OP.max, OP.min)

                sq = {}
                for j in SHARED_J + LEFT_ONLY + RIGHT_ONLY:
                    s = sqpool.tile([H, CHUNK], F32, tag="sq")
                    nc.scalar.activation(s[:], tc_[:], AF.Square,
                                         bias=jbias_sb[:, j:j + 1])
                    sq[j] = s

                g = 1
                nmul = 0

                def emit(moving, last=False):
                    nonlocal g
                    nc.tensor.matmul(out_ps[:], w_sb[:, g, :], moving[:],
                                     start=False, stop=last)
                    g += 1

                def mul(dst, a, b):
                    nonlocal nmul
                    eng = nc.gpsimd if (nmul % mulsplit == 0) else nc.vector
                    eng.tensor_tensor(dst[:], a[:], b[:], OP.mult)
                    nmul += 1

                for j in SHARED_J:
                    d3 = cubepool.tile([H, CHUNK], F32, tag="cube")
                    nc.vector.scalar_tensor_tensor(d3[:], tc_[:], float(j),
                                                   sq[j][:], OP.subtract, OP.mult)
                    emit(d3)
                    r = rspool.tile([H, CHUNK], F32, tag="r")
                    nc.vector.tensor_scalar(r[:], tc_[:], float(j), 0.0,
                                            OP.subtract, OP.max)
                    cr = cubepool.tile([H, CHUNK], F32, tag="cube")
                    mul(cr, r, sq[j])
                    emit(cr)
                for j in LEFT_ONLY:
                    r = rspool.tile([H, CHUNK], F32, tag="r")
                    nc.vector.tensor_scalar(r[:], tc_[:], float(j), 0.0,
                                            OP.subtract, OP.max)
                    cr = cubepool.tile([H, CHUNK], F32, tag="cube")
                    mul(cr, r, sq[j])
                    emit(cr)
                for j in RIGHT_ONLY:
                    m = rspool.tile([H, CHUNK], F32, tag="r")
                    nc.vector.tensor_scalar(m[:], tc_[:], float(j), 0.0,
                                            OP.subtract, OP.min)
                    cm = cubepool.tile([H, CHUNK], F32, tag="cube")
                    mul(cm, m, sq[j])
                    emit(cm, last=(j == RIGHT_ONLY[-1]))

            for c in range(n_chunks):
                xin = xpool.tile([H, CHUNK], F32, tag="xin")
                nc.sync.dma_start(xin[:], xt_d[:, c * CHUNK:(c + 1) * CHUNK])
                hT = psB.tile([H, CHUNK], F32, tag="h")
                kan_layer(xin, hT, w1_sb, mulsplit=3)
                embT = psE.tile([H, CHUNK], F32, tag="e")
                kan_layer(hT, embT, w2_sb, mulsplit=3)
                embT_sb = etpool.tile([H, CHUNK], F32, tag="embTsb")
                nc.vector.tensor_copy(embT_sb[:], embT[:])
                nc.sync.dma_start(embt_d[:, c * CHUNK:(c + 1) * CHUNK],
                                  embT_sb[:])

                idx = spool.tile([H, QQ, 1], F32, tag="idx")
                for q in range(QQ):
                    lhs = embT_sb[:, q * H:(q + 1) * H]
                    sim = psS.tile([H, K], F32, tag="sim")
                    nc.tensor.matmul(sim[:], lhs, pnt_sb[:], start=True, stop=True)
                    mx = spool.tile([H, 1], F32, tag="mx")
                    nc.vector.tensor_reduce(mx[:], sim[:], mybir.AxisListType.X,
                                            OP.max)
                    eq = aqpool.tile([H, K], F32, tag="eq")
                    nc.vector.tensor_scalar(eq[:], sim[:], mx[:], None, OP.is_equal)
                    msk = aqpool.tile([H, K], F32, tag="msk")
                    nc.vector.scalar_tensor_tensor(msk[:], eq[:], -BIG, iotab_sb[:],
                                                   OP.mult, OP.add)
                    nc.vector.tensor_reduce(idx[:, q, :], msk[:],
                                            mybir.AxisListType.X, OP.min)
                idxi = spool.tile([H, QQ], I32, tag="idxi")
                nc.vector.tensor_copy(idxi[:], idx[:, :, 0])
                nc.sync.dma_start(
                    asn_d[c * CHUNK:(c + 1) * CHUNK, :]
                    .rearrange("(q p) o -> p (q o)", p=H), idxi[:])
    nc.compile()
    return nc


_NC_CACHE = {}


def _get_nc(n_core):
    if n_core not in _NC_CACHE:
        _NC_CACHE[n_core] = build_kernel(n_core)
    return _NC_CACHE[n_core]


def _prep_consts(prototypes, grid, base_w1, spline_w1, scaler1,
                 base_w2, spline_w2, scaler2):
    w1 = _fold_weights(base_w1, spline_w1, scaler1)
    w2 = _fold_weights(base_w2, spline_w2, scaler2)
    pn = prototypes.astype(np.float32)
    norms = np.maximum(np.sqrt((pn.astype(np.float32) ** 2).sum(-1)), 1e-8)
    pnt = (pn / norms[:, None]).T.astype(np.float32).copy()  # (H, K)
    iotab = np.broadcast_to((np.arange(K, dtype=np.float32) + BIG)[None, :],
                            (H, K)).copy()
    jbias = np.broadcast_to(-np.arange(12, dtype=np.float32)[None, :],
                            (H, 12)).copy()
    return w1, w2, pnt, iotab, jbias


def kernel(x, prototypes, grid, base_w1, spline_w1, scaler1,
           base_w2, spline_w2, scaler2, _trace=False, _n_cores=N_CORES):
    x = np.asarray(x, dtype=np.float32)
    n = x.shape[0]
    n_core = n // _n_cores
    w1, w2, pnt, iotab, jbias = _prep_consts(
        np.asarray(prototypes), np.asarray(grid),
        np.asarray(base_w1), np.asarray(spline_w1), np.asarray(scaler1),
        np.asarray(base_w2), np.asarray(spline_w2), np.asarray(scaler2))
    nc = _get_nc(n_core)
    in_maps = []
    for i in range(_n_cores):
        in_maps.append({
            "xt": np.ascontiguousarray(x[i * n_core:(i + 1) * n_core].T),
            "w1": w1, "w2": w2, "pnt": pnt, "iotab": iotab, "jbias": jbias,
        })
    res = run_bass_kernel_spmd(nc, in_maps, core_ids=list(range(_n_cores)),
                               trace=_trace)
    emb = np.concatenate([np.ascontiguousarray(r["embt"].T)
                          for r in res.results], axis=0)
    asn = np.concatenate([r["asn"][:, 0] for r in res.results], axis=0)
    kernel._last_results = res
    return emb, asn.astype(np.int32)
